# revision 3
# baseline (speedup 1.0000x reference)
"""BiDAF attention-flow kernel for one TRN2 chip (8 NeuronCores).

Reference computation (per batch b):
    w1, w2, w3 = w[:D], w[D:2D], w[2D:]
    sim[c,q] = w1.C_c + w2.Q_q + w3.(C_c*Q_q)          # trilinear similarity
    c2q = softmax_q(sim) @ Q                            # [Lc, D]
    batt = softmax_c(max_q sim)                         # [Lc]
    q2c  = batt @ C, broadcast over Lc                  # [Lc, D]
    returns (c2q, q2c_broadcast)

Sharding: pure data parallel — batch 32 split 4-per-core over 8 cores, w
replicated.  No collectives.

Device algorithm per core (4 batches), all matmul inputs bf16, f32 PSUM
accumulation:
  - Load C, Q with f32->bf16 cast during SWDGE DMA.
  - PE-transpose Q -> QT, scale by w3 on evacuation, append w1 as an extra
    rhs column:  sim_psum[c, 0:128] = S3 + (K=1 ones-matmul adds s2 row),
    sim_psum[c, 128] = s1 (free).  Note softmax over q is invariant to the
    +s1[c] term, so it is only needed for the q2c branch.
  - ACT exp (no max subtraction needed; |sim| <~ 8) emits both exp(sim) and
    its row sums via accum_out.  c2q = (exp @ Q) scaled by 1/rowsum on PSUM
    evacuation.
  - m0[c] = max_q(sim) via DVE; z = exp(m0 + s1); q2c = (z @ C) / sum(z)
    using monotonicity of exp (max then exp == exp then max).
The q2c output is returned as [B, D] and broadcast to [B, Lc, D] on host
(pure replication — part of unsharding).
"""

import sys

for _p in ("/opt/trn_rl_repo", "/root/.axon_site/_ro/trn_rl_repo"):
    if _p not in sys.path:
        sys.path.append(_p)

from contextlib import ExitStack

import numpy as np

import concourse.bacc as bacc
import concourse.bass as bass
import concourse.tile as tile
from concourse import mybir
from concourse.bass_utils import run_bass_kernel_spmd
from concourse.masks import make_identity

F32 = mybir.dt.float32
BF16 = mybir.dt.bfloat16
AF = mybir.ActivationFunctionType
AX = mybir.AxisListType

B, LC, LQ, D = 32, 1024, 128, 1024
NCORES = 8
BPC = B // NCORES  # batches per core
NCT = LC // 128  # c-tiles per batch
NDT = D // 128  # d-tiles

_NC_CACHE = None


def build_kernel():
    nc = bacc.Bacc("TRN2", target_bir_lowering=False, debug=False, num_devices=NCORES)
    ctx_ext = nc.dram_tensor("ctx", [BPC, LC, D], F32, kind="ExternalInput").ap()
    q_ext = nc.dram_tensor("q", [BPC, LQ, D], F32, kind="ExternalInput").ap()
    w_ext = nc.dram_tensor("w", [3 * D], F32, kind="ExternalInput").ap()
    c2q_ext = nc.dram_tensor("c2q", [BPC, LC, D], F32, kind="ExternalOutput").ap()
    q2c_ext = nc.dram_tensor("q2c", [BPC, D], F32, kind="ExternalOutput").ap()

    with tile.TileContext(nc) as tc, ExitStack() as ctx:
        consts = ctx.enter_context(tc.tile_pool(name="consts", bufs=1))
        cn_pool = ctx.enter_context(tc.tile_pool(name="cn", bufs=2 * NCT))
        ct_pool = ctx.enter_context(tc.tile_pool(name="ct", bufs=2 * NDT))
        qn_pool = ctx.enter_context(tc.tile_pool(name="qn", bufs=2))
        qaug_pool = ctx.enter_context(tc.tile_pool(name="qaug", bufs=2))
        et_pool = ctx.enter_context(tc.tile_pool(name="et", bufs=4))
        ett_pool = ctx.enter_context(tc.tile_pool(name="ett", bufs=4))
        out_pool = ctx.enter_context(tc.tile_pool(name="outs", bufs=4))
        small = ctx.enter_context(tc.tile_pool(name="small", bufs=6))
        zp = ctx.enter_context(tc.tile_pool(name="zp", bufs=2))
        # PSUM: 8 banks total.  tags: tpose(2) + simish(2) + c2qp(2) + q2cp(2)
        tp_psum = ctx.enter_context(tc.tile_pool(name="tpose", bufs=2, space="PSUM"))
        sim_psum = ctx.enter_context(tc.tile_pool(name="simish", bufs=2, space="PSUM"))
        c2q_psum = ctx.enter_context(tc.tile_pool(name="c2qp", bufs=2, space="PSUM"))
        q2c_psum = ctx.enter_context(tc.tile_pool(name="q2cp", bufs=2, space="PSUM"))

        # ---- constants ----
        ident_bf = consts.tile([128, 128], BF16)
        make_identity(nc, ident_bf)
        ident_f32 = consts.tile([128, 128], F32)
        make_identity(nc, ident_f32)
        ones_row = consts.tile([1, 128], BF16)
        nc.vector.memset(ones_row, 1.0)
        ones_col = consts.tile([128, 1], BF16)
        nc.vector.memset(ones_col, 1.0)

        # ---- w prep: w1cols bf16 [128,8], w23cols=(w2/w3) bf16, w3cols f32 ----
        wsb = [consts.tile([NDT, 128], F32, tag=f"wsb{i}", name=f"wsb{i}") for i in range(3)]
        for i in range(3):
            nc.sync.dma_start(
                out=wsb[i],
                in_=w_ext[i * D : (i + 1) * D].rearrange("(a b) -> a b", b=128),
            )
        wps = []
        for i in range(3):
            wp = tp_psum.tile([128, NDT], F32, tag="tpose")
            nc.tensor.transpose(wp, wsb[i], ident_f32[:NDT, :NDT])
            wps.append(wp)
        w1cols = consts.tile([128, NDT], BF16)
        nc.vector.tensor_copy(w1cols, wps[0])
        w2cols = consts.tile([128, NDT], F32)
        nc.vector.tensor_copy(w2cols, wps[1])
        w3cols = consts.tile([128, NDT], F32)
        nc.vector.tensor_copy(w3cols, wps[2])
        w3inv = consts.tile([128, NDT], F32)
        nc.vector.reciprocal(w3inv, w3cols)
        w23cols = consts.tile([128, NDT], BF16)
        nc.vector.tensor_mul(w23cols, w2cols, w3inv)

        evac_flip = 0  # alternate DVE/ACT for PSUM evacuations

        for b in range(BPC):
            # ---- loads (cast f32 -> bf16 in DMA) ----
            qn = qn_pool.tile([LQ, D], BF16, tag="qn")
            nc.gpsimd.dma_start(out=qn, in_=q_ext[b])
            cn = []
            for ci in range(NCT):
                t = cn_pool.tile([128, D], BF16, tag="cn", name=f"cn{b}_{ci}")
                nc.gpsimd.dma_start(out=t, in_=ctx_ext[b, ci * 128 : (ci + 1) * 128])
                cn.append(t)

            # ---- Q prep: QT scaled by w3, plus w1 column ----
            qaug = qaug_pool.tile([128, NDT, 132], BF16, tag="qaug")
            for g in range(2):
                tp = tp_psum.tile([128, 512], BF16, tag="tpose")
                for k in range(4):
                    dt = 4 * g + k
                    nc.tensor.transpose(
                        tp[:, k * 128 : (k + 1) * 128],
                        qn[:, dt * 128 : (dt + 1) * 128],
                        ident_bf,
                    )
                for k in range(4):
                    dt = 4 * g + k
                    nc.vector.tensor_scalar_mul(
                        qaug[:, dt, 0:128],
                        tp[:, k * 128 : (k + 1) * 128],
                        w3cols[:, dt : dt + 1],
                    )
            for dt in range(NDT):
                nc.vector.tensor_copy(
                    qaug[:, dt, 128:129], w1cols[:, dt : dt + 1]
                )

            # ---- s2 row: s2[q] = sum_d w2 * QT = (w2/w3) . (w3*QT) ----
            s2p = sim_psum.tile([1, 128], F32, tag="simish")
            for dt in range(NDT):
                nc.tensor.matmul(
                    s2p,
                    w23cols[:, dt : dt + 1],
                    qaug[:, dt, 0:128],
                    start=(dt == 0),
                    stop=(dt == NDT - 1),
                )
            s2aug = qaug_pool.tile([1, 132], BF16, tag="s2aug")
            nc.vector.memset(s2aug, 0.0)
            nc.vector.tensor_copy(s2aug[:, 0:128], s2p)

            # ---- C transpose: CT[dt] = [128d, LC c] bf16 ----
            ctw = []
            for dt in range(NDT):
                t = ct_pool.tile([128, LC], BF16, tag="ct", name=f"ct{b}_{dt}")
                ctw.append(t)
            for dt in range(NDT):
                for g in range(2):
                    tp = tp_psum.tile([128, 512], BF16, tag="tpose")
                    for k in range(4):
                        ci = 4 * g + k
                        nc.tensor.transpose(
                            tp[:, k * 128 : (k + 1) * 128],
                            cn[ci][:, dt * 128 : (dt + 1) * 128],
                            ident_bf,
                        )
                    if evac_flip % 2 == 0:
                        nc.vector.tensor_copy(
                            ctw[dt][:, g * 512 : (g + 1) * 512], tp
                        )
                    else:
                        nc.scalar.copy(ctw[dt][:, g * 512 : (g + 1) * 512], tp)
                    evac_flip += 1

            # ---- per c-tile: sim, softmax pieces, c2q ----
            zcols = zp.tile([128, NCT], BF16, tag="zcols")
            q2cp = [
                q2c_psum.tile([1, 512], F32, tag="q2cp", name=f"q2cp{b}_{ch}")
                for ch in range(2)
            ]
            for ci in range(NCT):
                simp = sim_psum.tile([128, 132], F32, tag="simish")
                for dt in range(NDT):
                    nc.tensor.matmul(
                        simp[:, 0:129],
                        ctw[dt][:, ci * 128 : (ci + 1) * 128],
                        qaug[:, dt, 0:129],
                        start=(dt == 0),
                        stop=False,
                    )
                nc.tensor.matmul(
                    simp[:, 0:129],
                    ones_row,
                    s2aug[:, 0:129],
                    start=False,
                    stop=True,
                )
                m0 = small.tile([128, 1], F32, tag="m0")
                nc.vector.reduce_max(m0, simp[:, 0:128], axis=AX.X)
                s1c = small.tile([128, 1], F32, tag="s1c")
                nc.vector.tensor_copy(s1c, simp[:, 128:129])
                et = et_pool.tile([128, 128], BF16, tag="et")
                rsum = small.tile([128, 1], F32, tag="rsum")
                nc.scalar.activation(et, simp[:, 0:128], AF.Exp, accum_out=rsum)
                rinv = small.tile([128, 1], F32, tag="rinv")
                nc.vector.reciprocal(rinv, rsum)
                # z = exp(m0 + s1)
                nc.scalar.activation(
                    zcols[:, ci : ci + 1], m0, AF.Exp, bias=s1c
                )
                # transpose exp(sim) -> [q, c] for the c2q matmul
                etp = tp_psum.tile([128, 128], BF16, tag="tpose")
                nc.tensor.transpose(etp, et, ident_bf)
                ett = ett_pool.tile([128, 128], BF16, tag="ett")
                if evac_flip % 2 == 0:
                    nc.vector.tensor_copy(ett, etp)
                else:
                    nc.scalar.copy(ett, etp)
                evac_flip += 1
                # c2q = (exp(sim) @ Q) * rinv
                c2q_sb = out_pool.tile([128, D], F32, tag="c2q_sb")
                for ch in range(2):
                    cp = c2q_psum.tile([128, 512], F32, tag="c2qp")
                    nc.tensor.matmul(
                        cp,
                        ett,
                        qn[:, ch * 512 : (ch + 1) * 512],
                        start=True,
                        stop=True,
                    )
                    if ch == 0:
                        nc.vector.tensor_scalar_mul(
                            c2q_sb[:, ch * 512 : (ch + 1) * 512], cp, rinv
                        )
                    else:
                        nc.scalar.mul(
                            c2q_sb[:, ch * 512 : (ch + 1) * 512], cp, rinv
                        )
                nc.sync.dma_start(
                    out=c2q_ext[b, ci * 128 : (ci + 1) * 128], in_=c2q_sb
                )
                # q2c accumulation: sum_c z[c] * C[c, :]
                for ch in range(2):
                    nc.tensor.matmul(
                        q2cp[ch],
                        zcols[:, ci : ci + 1],
                        cn[ci][:, ch * 512 : (ch + 1) * 512],
                        start=(ci == 0),
                        stop=(ci == NCT - 1),
                    )

            # ---- q2c normalize: / sum(z) ----
            zs = tp_psum.tile([1, NCT], F32, tag="tpose")
            nc.tensor.matmul(zs, ones_col, zcols, start=True, stop=True)
            zsum = small.tile([1, 1], F32, tag="zsum")
            nc.vector.reduce_sum(zsum, zs, axis=AX.X)
            zrinv = small.tile([1, 1], F32, tag="zrinv")
            nc.vector.reciprocal(zrinv, zsum)
            q2c_sb = out_pool.tile([1, D], F32, tag="q2c_sb", name=f"q2c_sb{b}")
            for ch in range(2):
                nc.vector.tensor_scalar_mul(
                    q2c_sb[:, ch * 512 : (ch + 1) * 512],
                    q2cp[ch],
                    zrinv,
                )
            nc.sync.dma_start(out=q2c_ext[b : b + 1, :], in_=q2c_sb)

    nc.compile()
    return nc


def _get_nc():
    global _NC_CACHE
    if _NC_CACHE is None:
        _NC_CACHE = build_kernel()
    return _NC_CACHE


def kernel(context_features, question_features, w, _trace=False):
    nc = _get_nc()
    context_features = np.ascontiguousarray(context_features, dtype=np.float32)
    question_features = np.ascontiguousarray(question_features, dtype=np.float32)
    w = np.ascontiguousarray(w, dtype=np.float32)
    in_maps = []
    for core in range(NCORES):
        b0 = core * BPC
        in_maps.append(
            {
                "ctx": context_features[b0 : b0 + BPC],
                "q": question_features[b0 : b0 + BPC],
                "w": w,
            }
        )
    res = run_bass_kernel_spmd(
        nc, in_maps, core_ids=list(range(NCORES)), trace=_trace
    )
    c2q = np.concatenate([res.results[i]["c2q"] for i in range(NCORES)], axis=0)
    q2c_vec = np.concatenate([res.results[i]["q2c"] for i in range(NCORES)], axis=0)
    q2c = np.broadcast_to(q2c_vec[:, None, :], (B, LC, D))
    if _trace:
        kernel.last_exec_time_ns = res.exec_time_ns
    return (c2q, q2c)
